# revision 1
# baseline (speedup 1.0000x reference)
"""Single attention head (B=8, S=2048, D_IN=1024, D_OUT=64) on 8 TRN2 NeuronCores.

Strategy: pure data-parallel over batch -- core b computes batch element b's
full attention head. No collectives.

Per-core dataflow (v3 -- rebalanced so ScalarE's exp stream (the ~36.7us
wall: 32 x [128,1024] exps at (N+352)/1.2 ns) and the PE matmul stream
(~44us incl transposes) both run near-saturated):
  - Every input DMA is contiguous via host-side pre-layout (strided
    descriptors measured 4x slower).  seq (fp8 and bf16) is sliced into
    512-column sj chunks so compute starts after the first 512KB lands.
    All small constants pack into ONE uint8 buffer read back through
    bitcast APs -- each separate DMA costs ~2-3us of per-queue completion
    latency at startup.  The two seq streams alternate between the sync
    and scalar HWDGE queues (parallel rings; ScalarE's issue instructions
    all retire before its first exp).  Things measured and AVOIDED: SWDGE/
    gpsimd DMAs (~3.8us engine DRAIN each), DMA-engine xbar transposes
    (~1.2us issue each + head-of-line blocking), [1,N] single-partition
    DVE ops (reciprocal on [1,512] = 3.35us vs 0.17us for [128,1]).
  - K/Q projections in fp8 DoubleRow (x32-scaled stacked [wk|wq] / [wq|wk]
    weights in [p, c, i, f] interleave, bias via one DVE drain each, the
    x1024 folded into the exp scale).  fp8 q/k measures 3.2e-3 overall rel
    err vs the f32 reference (gate 2e-2); V must stay bf16.
  - scores: key-chunk pairs row-tiled on the PE (rows 0:64 / 64:128 co-run)
    into [128, 1024] psum pair tiles; ONE exp per pair tile (no
    max-subtraction: |scores/sqrt(S)| << 1).  The mask is NOT applied to
    scores: masked keys' V rows and the ones-column (denominator) are
    zeroed instead, which is exact.
  - schedule: qc-major.  Phase A streams q-chunk 0's 8 pair blocks inside
    the KQ projection loop; phases B/C/D emit one pair block per exp cycle
    (~1.15us) with PE filler sized to the slack: V-projection units in B,
    two deferred ctx matmuls per block in C/D.  Every streamed tensor
    (kqT/kq2T/vT/v/out) is split into per-sj or per-qc TILES -- coarse
    tile-granular dependencies otherwise gate early readers on the last
    writer (measured: phase-A pairs stalled until the sj3 drain).
  - ctx matmuls accumulate ctxT[65, q] in psum (row 64 = keep-mask dot exp
    = softmax denominator); finalize per q-chunk fires the moment its last
    ctx matmul retires: drain, PE-transpose back to [q, 65], multiply rows
    by 1/ctx[.., 64], DMA out -- all overlapping the remaining exp stream.
  - 2 junk warmup matmuls pull the PE HAM clock-gate toward 8/8 (2.4GHz)
    before the first real matmul; the dense schedule then keeps it warm
    (idle >3.4us re-throttles to 1.2GHz).
Known residual losses (run-to-run chip power state adds +-10us on top):
~5us exp-stream idle in phase A (the list scheduler's optimistic DMA model
orders sj2/sj3 KQ and V units ahead of the phase-A pair blocks; priority /
wait_until overrides measured worse), ~7us serial tail (last exp -> last
ctx pops -> finalize(3) -> output DMA flush -> drain barrier).
"""

import numpy as np
import ml_dtypes

import concourse.bass as bass  # noqa: F401  (bass types used via tile/bacc)
import concourse.mybir as mybir
import concourse.tile as tile
from concourse import bacc
from concourse.bass_utils import run_bass_kernel_spmd

B, S, D, F = 8, 2048, 1024, 64
NCORES = 8
BF = mybir.dt.bfloat16
F8 = mybir.dt.float8e4
F32 = mybir.dt.float32
# reference scales by sqrt(S); q and k each carry x32 from the fp8 weight scaling
SCALE = 1.0 / (1024.0 * float(np.sqrt(np.float32(S))))
SC = 512  # matmul moving free-dim
NSJ = S // SC  # 4 column chunks of the projection loop
KCH = S // 128  # 16 key chunks
DCH = D // 128  # 8 bf16 contraction chunks
DR = D // 256  # 4 fp8 DoubleRow contraction chunks
CONSTS_B = 3916  # packed consts bytes per partition


def _emit(nc):
    # all layouts pre-arranged host-side so every DMA below is contiguous
    seqf8_d = nc.declare_dram_parameter("seqf8", [NSJ, 128, DR, 2, SC], F8, isOutput=False)
    seqb_d = nc.declare_dram_parameter("seqb", [NSJ, 128, DCH, SC], BF, isOutput=False)
    # all small constants pack into ONE byte buffer -> ONE DMA (each extra
    # DMA costs ~2-3us of per-queue completion latency at startup):
    # [0:1024) wkq f8, [1024:2048) wqk f8, [2048:3072) wv bf16,
    # [3072:3148) misc f32 (19 cols: biases + keep-mask),
    # [3148:3404) identb bf16, [3404:3916) identf f32
    consts_d = nc.declare_dram_parameter("consts", [128, CONSTS_B], mybir.dt.uint8, isOutput=False)
    out_d = nc.declare_dram_parameter("out", [S, F], F32, isOutput=True)

    with tile.TileContext(nc) as tc:
        _body(nc, tc, seqf8_d, seqb_d, consts_d, out_d)
    nc.compile()


def _body(nc, tc, seqf8_d, seqb_d, consts_d, out_d):
    from contextlib import ExitStack

    with ExitStack() as ctx:
        const = ctx.enter_context(tc.tile_pool(name="const", bufs=1))
        big = ctx.enter_context(tc.tile_pool(name="big", bufs=1))
        sbw = ctx.enter_context(tc.tile_pool(name="sbw", bufs=1))
        ps = ctx.enter_context(tc.tile_pool(name="ps", space="PSUM", bufs=1))

        # ---- input DMAs on TWO HWDGE queues.  sync: the seq streams,
        # ordered by first-use time (each DMA pays ~2.4us of transfer +
        # completion latency, serialized per queue).  scalar: the single
        # packed consts DMA (ScalarE is otherwise idle until the first
        # exp, and the scalar HWDGE ring runs parallel to sync's) ----
        seqf8 = [
            big.tile([128, DR, 2, SC], F8, name=f"seqf8_{j}") for j in range(NSJ)
        ]
        # seqb split per HALF-chunk (c 0:4 / 4:8 of each sj): a V unit only
        # contracts over one half, and the 512KB granularity lands ~3-4us
        # earlier on the bandwidth-bound queues than full 1MB chunks
        seqbh = [
            big.tile([128, DCH // 2, SC], BF, name=f"seqb_{j}_{h}")
            for j in range(NSJ)
            for h in range(2)
        ]
        consts_sb = const.tile([128, CONSTS_B], mybir.dt.uint8, name="consts_sb")
        # Both HWDGE queues stream in parallel; each DMA costs ~2.4us of
        # in-queue slot (transfer + completion latency), so the two seq
        # streams alternate queues in first-use order.  consts leads the
        # sync queue; ScalarE's issue instructions all retire before its
        # first exp.
        nc.sync.dma_start(out=consts_sb[:], in_=consts_d.ap())
        nc.scalar.dma_start(out=seqf8[0][:], in_=seqf8_d[0])
        nc.sync.dma_start(out=seqf8[1][:], in_=seqf8_d[1])
        nc.scalar.dma_start(out=seqf8[2][:], in_=seqf8_d[2])
        nc.sync.dma_start(out=seqf8[3][:], in_=seqf8_d[3])
        for j, h, eng in ((0, 0, nc.scalar), (0, 1, nc.sync), (1, 0, nc.scalar),
                          (1, 1, nc.sync), (2, 0, nc.scalar), (2, 1, nc.sync),
                          (3, 0, nc.scalar), (3, 1, nc.sync)):
            eng.dma_start(
                out=seqbh[2 * j + h][:],
                in_=seqb_d[j, :, 4 * h : 4 * h + 4, :],
            )
        wkq_sb = consts_sb[:, 0:1024].bitcast(F8).rearrange(
            "p (c i f) -> p c i f", c=DR, i=2
        )
        wqk_sb = consts_sb[:, 1024:2048].bitcast(F8).rearrange(
            "p (c i f) -> p c i f", c=DR, i=2
        )
        wv_sb = consts_sb[:, 2048:3072].bitcast(BF).rearrange(
            "p (c f) -> p c f", c=DCH
        )
        misc_sb = consts_sb[:, 3072:3148].bitcast(F32)
        identb_sb = consts_sb[:, 3148:3404].bitcast(BF)
        identf_sb = consts_sb[:, 3404:3916].bitcast(F32)

        # preload the exp table set so the ~2.7us table DMA overlaps the loads
        dummy_sb = const.tile([1, 1], F32, name="dummy_sb")
        nc.scalar.activation(
            out=dummy_sb[:],
            in_=misc_sb[0:1, 0:1],
            func=mybir.ActivationFunctionType.Exp,
            scale=1.0,
        )

        # kqT: k on rows 0:64 (pair A lhsT), q on rows 64:128 (pair B rhs)
        # kq2T (reversed stacking): q on rows 0:64 (pair A rhs), k on rows
        # 64:128 (pair B lhsT).  All streamed tensors are split into per-sj
        # (or per-qc) TILES: tile-granular dependency tracking otherwise
        # gates readers of early columns on the LAST writer of the tensor
        # (measured: phase-A pair blocks stalled until the sj3 drain)
        kqT = [big.tile([128, SC], BF, name=f"kqT_{j}") for j in range(NSJ)]
        kq2T = [big.tile([128, SC], BF, name=f"kq2T_{j}") for j in range(NSJ)]
        vT = [big.tile([F, SC], BF, name=f"vT_{j}") for j in range(NSJ)]
        v_sbs = [big.tile([128, 4, F + 1], BF, name=f"v_sb{j}") for j in range(NSJ)]
        out_sbs = [big.tile([128, 4, F], F32, name=f"out_sb{q}") for q in range(4)]
        out_r = out_d.ap().rearrange("(c p) f -> p c f", p=128)

        bkq_ap = misc_sb[:, 0:1]  # stacked 32*[bk; bq]
        bqk_ap = misc_sb[:, 1:2]  # stacked 32*[bq; bk]
        bv_ap = misc_sb[0:F, 2:3]
        mask01 = misc_sb[:, 3:]  # [128, 16] 1.0 = keep, 0.0 = masked out

        # ones-column of v := keep-mask (masked keys contribute 0 to the sums)
        for j in range(NSJ):
            nc.vector.tensor_copy(v_sbs[j][:, :, F], mask01[:, 4 * j : 4 * j + 4])

        # HAM warmup: junk matmuls on the consts buffer while seqf8[0] is in
        # flight, pulling the PE clock-gate toward 8/8 before real work
        warm_rhs = consts_sb[:, 0:SC].bitcast(F8)
        for i in range(2):
            ps_warm = ps.tile([128, SC], F32, tag="pk", bufs=2, name=f"ps_warm{i}")
            nc.tensor.matmul(
                ps_warm[:], wkq_sb[:, 0, 0, :], warm_rhs, start=True, stop=True
            )

        ctx_tiles = {}
        pending_ctx = []  # deferred ctx matmuls -- popped as PE filler

        def emit_ctx(qc, p, expq):
            ctx_ps = ctx_tiles[qc]
            ka, kb = 2 * p, 2 * p + 1
            nc.tensor.matmul(
                ctx_ps[:],
                v_sbs[ka // 4][:, ka % 4, :],
                expq[:, 0:SC],
                start=(p == 0),
                stop=False,
            )
            nc.tensor.matmul(
                ctx_ps[:],
                v_sbs[kb // 4][:, kb % 4, :],
                expq[:, SC : 2 * SC],
                start=False,
                stop=(p == KCH // 2 - 1),
            )

        def pop_ctx(n):
            for _ in range(min(n, len(pending_ctx))):
                qc, p, expq = pending_ctx.pop(0)
                emit_ctx(qc, p, expq)
                if p == KCH // 2 - 1:
                    # that was qc's last ctx matmul -- drain it now so its
                    # psum slot frees up and the output DMA overlaps
                    finalize(qc)

        def pair_block(qc, p):
            # scores for key chunks (2p, 2p+1) x q-chunk qc, then exp.
            if qc not in ctx_tiles:
                ctx_tiles[qc] = ps.tile(
                    [F + 1, SC], F32, tag="ctx", bufs=2, name=f"ctx_ps{qc}"
                )
            ka, kb = 2 * p, 2 * p + 1
            ps_pair = ps.tile(
                [128, 2 * SC], F32, tag="pair", bufs=2, name=f"ps_pair_{qc}_{p}"
            )
            # chunk A on array rows 0:64, chunk B on rows 64:128 --
            # disjoint row groups run concurrently on the PE
            nc.tensor.matmul(
                ps_pair[:, 0:SC],
                kqT[ka // 4][0:F, (ka % 4) * 128 : (ka % 4 + 1) * 128],
                kq2T[qc][0:F, :],
                start=True,
                stop=True,
            )
            nc.tensor.matmul(
                ps_pair[:, SC : 2 * SC],
                kq2T[kb // 4][64:128, (kb % 4) * 128 : (kb % 4 + 1) * 128],
                kqT[qc][64:128, :],
                start=True,
                stop=True,
            )
            expq = sbw.tile(
                [128, 2 * SC], BF, tag="expq", bufs=16, name=f"expq_{qc}_{p}"
            )
            nc.scalar.activation(
                out=expq[:],
                in_=ps_pair[:],
                func=mybir.ActivationFunctionType.Exp,
                scale=SCALE,
            )
            pending_ctx.append((qc, p, expq))

        # ---- V-projection filler units (PE work to fill exp-paced slack) ----
        vps = {}

        def v_unit(u):
            sj, h = divmod(u, 2)
            if h == 0:
                vps[sj] = ps.tile([F, SC], F32, tag="pk", bufs=2, name=f"ps_v{sj}")
            for c in range(4 * h, 4 * h + 4):
                nc.tensor.matmul(
                    vps[sj][:],
                    wv_sb[:, c, :],
                    seqbh[2 * sj + h][:, c - 4 * h, :],
                    start=(c == 0),
                    stop=(c == DCH - 1),
                )
            if h == 1:
                nc.vector.tensor_scalar_add(vT[sj][:], vps[sj][:], bv_ap)
                # transpose into natural [k, f] layout on the PE
                for i in range(4):
                    t = 4 * sj + i
                    vtp = ps.tile([128, F], BF, tag="pk", bufs=2, name=f"vtp{t}")
                    nc.tensor.transpose(
                        vtp[:],
                        vT[sj][:, i * 128 : (i + 1) * 128],
                        identb_sb[0:F, 0:F],
                    )
                    nc.vector.tensor_scalar_mul(
                        v_sbs[sj][:, i, 0:F], vtp[:], mask01[:, t : t + 1]
                    )

        def finalize(qc):
            # drain ctx, reciprocal the denominator row IN the transposed-
            # domain copy, PE-transpose back to [q, 65] (col 64 = 1/den),
            # scale rows, store
            ctx_ps = ctx_tiles.pop(qc)
            ctxTq = sbw.tile([F + 1, SC], F32, tag="ctxTq", bufs=2, name=f"ctxTq{qc}")
            nc.vector.tensor_copy(ctxTq[:], ctx_ps[:])
            for i in range(SC // 128):
                t = qc * 4 + i
                ctp = ps.tile([128, F + 1], F32, tag="pk", bufs=2, name=f"ctp{t}")
                nc.tensor.transpose(
                    ctp[:],
                    ctxTq[:, i * 128 : (i + 1) * 128],
                    identf_sb[0 : F + 1, 0 : F + 1],
                )
                rec = sbw.tile([128, 1], F32, tag="rec", bufs=4, name=f"rec{t}")
                nc.vector.reciprocal(rec[:], ctp[:, F : F + 1])
                nc.vector.tensor_scalar_mul(
                    out_sbs[qc][:, i, :], ctp[:, 0:F], rec[:]
                )
            nc.sync.dma_start(
                out=out_r[:, qc * 4 : (qc + 1) * 4, :],
                in_=out_sbs[qc][:],
            )

        # ---- Phase A        # ---- Phase A        # ---- Phase A: K/Q projections with q-chunk 0's pair blocks ----
        for sj in range(NSJ):
            ps_kq = ps.tile([128, SC], F32, tag="pk", bufs=2, name=f"ps_kq{sj}")
            ps_kq2 = ps.tile([128, SC], F32, tag="pk", bufs=2, name=f"ps_kq2_{sj}")
            for c in range(DR):
                rhs = seqf8[sj][:, c, :, :]
                st = dict(start=(c == 0), stop=(c == DR - 1))
                nc.tensor.matmul(
                    ps_kq[:], wkq_sb[:, c, :, :], rhs,
                    perf_mode=mybir.MatmulPerfMode.DoubleRow, **st
                )
                nc.tensor.matmul(
                    ps_kq2[:], wqk_sb[:, c, :, :], rhs,
                    perf_mode=mybir.MatmulPerfMode.DoubleRow, **st
                )
            nc.vector.tensor_scalar_add(kqT[sj][:], ps_kq[:], bkq_ap)
            nc.vector.tensor_scalar_add(kq2T[sj][:], ps_kq2[:], bqk_ap)
            pair_block(0, 2 * sj)
            pair_block(0, 2 * sj + 1)

        # ---- Phases B/C/D: qc 1..3 pair blocks, exp-paced.  Per-block PE
        # budget ~1.15us (one exp): pair (~220ns co-run) + one V unit
        # (~850ns) or two deferred ctx pops (~850ns).  V units 6/7 slide
        # into phase C so they don't outrun the bf16 seq stream ----
        for p in range(KCH // 2):  # qc = 1
            pair_block(1, p)
            if p < 6:
                v_unit(p)
            else:
                pop_ctx(2)
        for p in range(KCH // 2):  # qc = 2
            pair_block(2, p)
            if p < 2:
                v_unit(6 + p)
                pop_ctx(1)
            else:
                pop_ctx(2)
        for p in range(KCH // 2):  # qc = 3
            pair_block(3, p)
            pop_ctx(2)
        pop_ctx(len(pending_ctx))
        pop_ctx(len(pending_ctx))


_NC_CACHE = None


def _get_nc():
    global _NC_CACHE
    if _NC_CACHE is None:
        nc = bacc.Bacc("TRN2", target_bir_lowering=False, debug=False)
        _emit(nc)
        _NC_CACHE = nc
    return _NC_CACHE


def make_in_maps(seq, mask, Wq, bq, Wk, bk, Wv, bv):
    bf16 = ml_dtypes.bfloat16
    f8 = ml_dtypes.float8_e4m3
    seq = np.asarray(seq, dtype=np.float32)
    mask = np.asarray(mask).astype(bool)
    wkq = np.concatenate(
        [np.asarray(Wk, dtype=np.float32), np.asarray(Wq, dtype=np.float32)], axis=1
    )  # [D, 128]
    wqk = np.concatenate(
        [np.asarray(Wq, dtype=np.float32), np.asarray(Wk, dtype=np.float32)], axis=1
    )
    # DoubleRow layout [p, c, i, f] for row index d = 256c + 2p + i, contiguous
    wkq_h = np.ascontiguousarray(
        (wkq * 32.0).astype(f8).reshape(DR, 128, 2, 128).transpose(1, 0, 2, 3)
    )
    wqk_h = np.ascontiguousarray(
        (wqk * 32.0).astype(f8).reshape(DR, 128, 2, 128).transpose(1, 0, 2, 3)
    )
    wv_h = np.ascontiguousarray(
        np.asarray(Wv, dtype=np.float32).astype(bf16).reshape(DCH, 128, F).transpose(1, 0, 2)
    )
    consts = np.zeros((NCORES, 128, CONSTS_B), dtype=np.uint8)
    consts[:, :, 0:1024] = wkq_h.reshape(128, 1024).view(np.uint8)
    consts[:, :, 1024:2048] = wqk_h.reshape(128, 1024).view(np.uint8)
    consts[:, :, 2048:3072] = wv_h.reshape(128, 512).view(np.uint8)
    consts[:, :, 3148:3404] = np.eye(128, dtype=bf16).view(np.uint8)
    consts[:, :, 3404:3916] = np.eye(128, dtype=np.float32).view(np.uint8)
    in_maps = []
    for b in range(NCORES):
        seqT = np.ascontiguousarray(seq[b].T)  # [D, S] f32
        # fp8, sliced by sj: [sj, p, c, i, t]
        sf8 = np.ascontiguousarray(
            seqT.astype(f8).reshape(DR, 128, 2, NSJ, SC).transpose(3, 1, 0, 2, 4)
        )
        # bf16, sliced by sj: [sj, p, c, t]
        sb16 = np.ascontiguousarray(
            seqT.astype(bf16).reshape(DCH, 128, NSJ, SC).transpose(2, 1, 0, 3)
        )
        misc = np.zeros((128, 3 + KCH), dtype=np.float32)
        misc[0:F, 0] = 32.0 * np.asarray(bk, dtype=np.float32)
        misc[64:128, 0] = 32.0 * np.asarray(bq, dtype=np.float32)
        misc[0:F, 1] = 32.0 * np.asarray(bq, dtype=np.float32)
        misc[64:128, 1] = 32.0 * np.asarray(bk, dtype=np.float32)
        misc[0:F, 2] = np.asarray(bv, dtype=np.float32)
        # keep-mask: misc[p, 3+c] = 0.0 if key c*128+p is masked out else 1.0
        misc[:, 3:] = np.where(mask[b], np.float32(0.0), np.float32(1.0)).reshape(
            KCH, 128
        ).T
        consts[b, :, 3072:3148] = misc.view(np.uint8)
        in_maps.append(
            {
                "seqf8": sf8,
                "seqb": sb16,
                "consts": consts[b],
            }
        )
    return in_maps


def run(in_maps, trace=False, **kw):
    nc = _get_nc()
    return run_bass_kernel_spmd(
        nc, in_maps, core_ids=list(range(NCORES)), trace=trace, **kw
    )


def kernel(seq, mask, Wq, bq, Wk, bk, Wv, bv):
    in_maps = make_in_maps(seq, mask, Wq, bq, Wk, bk, Wv, bv)
    res = run(in_maps)
    out = np.stack(
        [np.asarray(res.results[i]["out"], dtype=np.float32) for i in range(NCORES)],
        axis=0,
    )
    return out

